# revision 16
# baseline (speedup 1.0000x reference)
"""Trainium2 Bass kernel: adaptive-input softmax ('softmax' mode), 8 NeuronCores.

Strategy: vocab tensor-parallel. Each core owns a 1/8 slice of the head token
columns (2500 of 20000), tail0 columns (2500 of 20000) and tail1 columns
(1283 of ceil(10257/8)*8, zero-padded), computes partition-local logits ->
exp, and the per-row softmax denominators are completed with a small
cross-core AllGather of per-row exp-sums (overlapped with compute).
The 2 cluster logits are computed replicated on every core inside the wide
head exp; their contribution is rescaled (x1/8) on the DVE before the
AllGather so the 8-way sum reconstructs them exactly once.

Precision plan: the head matmul (99.99% of output probability mass) runs in
bf16. The tail projections and the tail0 logit matmul run in fp8e4m3 with
DoubleRow perf mode; the tails carry ~1e-4 of the probability mass so their
~1% internal error is invisible in the rel-l2 metric. Scale management:
host stores p0*16, p1*16, w0*8 in fp8; the kernel stores h0/8 in fp8 so
h0/8 @ (8*w0) reproduces h0@w0 exactly. Output stores in bf16.

v2 perf structure (from the v1 trace):
- PSUM chunks are 1024 wide (2 banks) with a 3-deep ring: 8 exp ACTIVATEs
  per m-tile instead of 13 -> ~70us less ACT time (ACTIVATE overhead +
  ACTIVATION_READ_ACCUMULATOR drains).
- Loads are ~20 coarse DMAs (3D dram params) in first-use order across the
  sync/scalar/gpsimd queues instead of ~130 fine ones; first matmul ~5us.
- Block sizes [3,3,3,3,2,1,1]: the last AllGathers carry 1 m-tile each, so
  after the final matmul only one small AG (~12us) + one m-tile of
  normalize+store remains.
- proj PSUM->SBUF copies ride the DVE, keeping ACT for exps.
"""
import numpy as np
import ml_dtypes
from contextlib import ExitStack

import concourse.bass as bass
import concourse.tile as tile
from concourse import bacc, mybir
from concourse.bass_utils import run_bass_kernel_spmd
from concourse.masks import make_identity

N_CORES = 8
D = 1024
KT = D // 128                      # contraction k-tiles over D
B0 = 20000                         # head token columns
HEAD_SLICE = B0 // N_CORES         # 2500 per core
T0_ALL = 20000
T0_SLICE = T0_ALL // N_CORES       # 2500 per core
T1_ALL = 10257
T1_SLICE = -(-T1_ALL // N_CORES)   # 1283 per core (global pad to 10264)
T1_PADDED = T1_SLICE * N_CORES
HEAD_COLS = HEAD_SLICE + 2         # + 2 replicated cluster columns
OUT_COLS = HEAD_SLICE + T0_SLICE + T1_SLICE   # 6283 per-core output columns
P0 = 256                           # tail0 projection dim
P1 = 64                            # tail1 projection dim
V = B0 + T0_ALL + T1_ALL           # 50257
SW = 16.0                          # head-weight fp8 pre-scale (tails)
CW = 1024                          # psum chunk width (2 banks)
F8C = 1024                         # head cols per core computed in fp8 (DR)
BF_COLS = HEAD_COLS - F8C          # remaining bf16 head cols (incl clusters)

BF16 = mybir.dt.bfloat16
F32 = mybir.dt.float32
F8 = mybir.dt.float8e4
DR = mybir.MatmulPerfMode.DoubleRow
EXP = mybir.ActivationFunctionType.Exp
COPY = mybir.ActivationFunctionType.Copy
ADD = mybir.AluOpType.add
AX = mybir.AxisListType.X


def _tiles(total, step=512):
    out, off = [], 0
    while off < total:
        w = min(step, total - off)
        out.append((off, w))
        off += w
    return out


def build(rows):
    assert rows % 512 == 0
    m_tiles = rows // 128

    nc = bacc.Bacc("TRN2", target_bir_lowering=False, debug=False,
                   num_devices=N_CORES)
    # 3D layouts so each logical tensor loads in one (or per-k) DMA.
    xT_ext = nc.declare_dram_parameter("xT", [128, KT, rows], BF16, isOutput=False)
    x8_ext = nc.declare_dram_parameter("x8", [128, KT, rows], F8, isOutput=False)
    wh_ext = nc.declare_dram_parameter("wh", [128, KT, BF_COLS], BF16, isOutput=False)
    wh8_ext = nc.declare_dram_parameter("wh8", [128, KT, F8C], F8, isOutput=False)
    p0_ext = nc.declare_dram_parameter("p0", [128, KT, P0], F8, isOutput=False)
    w0_ext = nc.declare_dram_parameter("w0", [128, P0 // 128, T0_SLICE], F8, isOutput=False)
    p1_ext = nc.declare_dram_parameter("p1", [128, KT, P1], F8, isOutput=False)
    w1_ext = nc.declare_dram_parameter("w1", [P1, T1_SLICE], BF16, isOutput=False)
    npad_ext = nc.declare_dram_parameter("negpad", [128, 1], F32, isOutput=False)
    out_ext = nc.declare_dram_parameter("out", [rows, OUT_COLS], BF16, isOutput=True)

    # head chunk 0 (F8C cols) runs fp8+DoubleRow; the rest is bf16
    head_chunks = _tiles(HEAD_COLS, CW)   # (0,1024),(1024,1024),(2048,454)
    t0_chunks = _tiles(T0_SLICE, CW)      # (0,1024),(1024,1024),(2048,452)
    t1_chunks = _tiles(T1_SLICE, CW)      # (0,1024),(1024,259)
    nh, n0, n1 = len(head_chunks), len(t0_chunks), len(t1_chunks)
    NPART = nh + n0 + n1                  # partial-sum columns per m-tile
    proj_tiles = _tiles(rows, 512)
    out_chunks = _tiles(OUT_COLS, -(-OUT_COLS // 3))   # 3 even-ish store chunks
    CH = out_chunks[0][1]

    with ExitStack() as ctx:
        tc = ctx.enter_context(tile.TileContext(nc))
        const = ctx.enter_context(tc.tile_pool(name="const", bufs=1))
        psum_pool = ctx.enter_context(tc.tile_pool(name="psum", bufs=3, space="PSUM"))
        psum_a = ctx.enter_context(tc.tile_pool(name="psum_a", bufs=1, space="PSUM"))
        psum_g = ctx.enter_context(tc.tile_pool(name="psum_g", bufs=1, space="PSUM"))
        exp_pool = ctx.enter_context(tc.tile_pool(name="exppool", bufs=6))
        outp = ctx.enter_context(tc.tile_pool(name="outp", bufs=3))
        small = ctx.enter_context(tc.tile_pool(name="small", bufs=7))
        dram = ctx.enter_context(tc.tile_pool(name="dram", bufs=3, space="DRAM"))

        # ---------- resident inputs ----------
        xT_sb = const.tile([128, KT, rows], BF16, name="xT_sb")
        x8_sb = const.tile([128, KT, rows], F8, name="x8_sb")
        wh_sb = const.tile([128, KT, BF_COLS], BF16, name="wh_sb")
        wh8_sb = const.tile([128, KT, F8C], F8, name="wh8_sb")
        p0_sb = const.tile([128, KT, P0], F8, name="p0_sb")
        p1_sb = const.tile([128, KT, P1], F8, name="p1_sb")
        w0_sb = const.tile([128, P0 // 128, T0_SLICE], F8, name="w0_sb")
        w1_sb = const.tile([P1, T1_SLICE], BF16, name="w1_sb")
        npad_sb = const.tile([128, 1], F32, name="npad_sb")

        # warm-up AllGather first, with no dependencies at all, so the
        # ~40us first-collective init burns on the CC engine starting at
        # t~1us, fully inside the load phase
        warm_sb = small.tile([1, 128], F32, name="warm_sb")
        nc.gpsimd.memset(warm_sb[:, :], 0.0)
        warm_in = dram.tile([1, 128], F32, name="warm_in")
        warm_out = dram.tile([N_CORES, 128], F32, name="warm_out",
                             addr_space="Shared")
        nc.gpsimd.dma_start(out=warm_in[:, :], in_=warm_sb[:, :])
        nc.gpsimd.collective_compute(
            "AllGather", mybir.AluOpType.bypass,
            replica_groups=[list(range(N_CORES))],
            ins=[warm_in.opt()], outs=[warm_out.opt()],
        )

        # ---------- loads: one HWDGE queue, strict first-use order ----------
        # A single logical DMA queue drains roughly in order, so transfers
        # complete in the order compute consumes them: m0's fp8 head chunk
        # first, then the bf16 head weights per-k, then tail weights, then
        # the remaining x row-chunks (needed from m-tile 4 on).
        nc.sync.dma_start(out=x8_sb[:, :, 0:512], in_=x8_ext[:, :, 0:512])
        nc.sync.dma_start(out=wh8_sb[:, :, :], in_=wh8_ext[:, :, :])
        nc.sync.dma_start(out=xT_sb[:, :, 0:512], in_=xT_ext[:, :, 0:512])
        for k in range(KT):
            nc.sync.dma_start(out=wh_sb[:, k, :], in_=wh_ext[:, k, :])
        nc.sync.dma_start(out=p0_sb[:, :, :], in_=p0_ext[:, :, :])
        nc.sync.dma_start(out=p1_sb[:, :, :], in_=p1_ext[:, :, :])
        nc.sync.dma_start(out=w0_sb[:, :, :], in_=w0_ext[:, :, :])
        nc.sync.dma_start(out=w1_sb[:, :], in_=w1_ext[:, :])
        nc.sync.dma_start(out=npad_sb[:], in_=npad_ext[:])
        for roff in range(512, rows, 512):
            nc.sync.dma_start(out=xT_sb[:, :, roff:roff + 512],
                              in_=xT_ext[:, :, roff:roff + 512])
            nc.sync.dma_start(out=x8_sb[:, :, roff:roff + 512],
                              in_=x8_ext[:, :, roff:roff + 512])

        ident = const.tile([128, 128], F32, name="ident")
        make_identity(nc, ident)

        # ---------- tail hidden projections (fp8 DoubleRow) ----------
        # h0T8 holds h0/8 in fp8: psum = x8 @ (16 p0) = 16 h0, DVE scale
        # 1/128 -> h0/8.  h1T holds h1 in bf16 (psum = 16 h1, scale 1/16).
        h0T8_sb = const.tile([128, P0 // 128, rows], F8, name="h0T8_sb")
        h1T_sb = const.tile([P1, rows], BF16, name="h1T_sb")

        def emit_proj(rt):
            roff, rw = proj_tiles[rt]
            for mp in range(P0 // 128):
                ps = psum_pool.tile([128, CW], F32, name="ps")
                for kp in range(KT // 2):
                    nc.tensor.matmul(ps[:, :rw],
                                     lhsT=p0_sb[:, 2 * kp:2 * kp + 2, mp * 128:(mp + 1) * 128],
                                     rhs=x8_sb[:, 2 * kp:2 * kp + 2, roff:roff + rw],
                                     start=(kp == 0), stop=(kp == KT // 2 - 1),
                                     perf_mode=DR)
                nc.vector.tensor_scalar_mul(h0T8_sb[:, mp, roff:roff + rw],
                                            ps[:, :rw], 1.0 / 128.0)
            ps = psum_pool.tile([128, CW], F32, name="ps")
            for kp in range(KT // 2):
                nc.tensor.matmul(ps[:P1, :rw],
                                 lhsT=p1_sb[:, 2 * kp:2 * kp + 2, :],
                                 rhs=x8_sb[:, 2 * kp:2 * kp + 2, roff:roff + rw],
                                 start=(kp == 0), stop=(kp == KT // 2 - 1),
                                 perf_mode=DR)
            nc.vector.tensor_scalar_mul(h1T_sb[:, roff:roff + rw],
                                        ps[:P1, :rw], 1.0 / 16.0)

        # ---------- per m-tile compute (head -> proj -> tails) ----------
        n_rt = len(proj_tiles)

        def emit_compute(m, j, sums_blk, per_m):
            r0 = m * 128
            exph = exp_pool.tile([128, HEAD_COLS], BF16, name="exph")
            expt0 = exp_pool.tile([128, T0_SLICE], BF16, name="expt0")
            expt1 = exp_pool.tile([128, T1_SLICE], BF16, name="expt1")
            partials = small.tile([128, NPART], F32, name="partials")
            j3 = j * 3

            pcol = 0
            for hi, (off, w) in enumerate(head_chunks):
                ps = psum_pool.tile([128, CW], F32, name="ps")
                small_chunk = (hi == nh - 1)
                if off < F8C:
                    # fp8 DoubleRow head chunk: psum = x8 @ (16*wh8)
                    assert off + w <= F8C
                    for soff, sw in _tiles(w, 512):
                        for kp in range(KT // 2):
                            nc.tensor.matmul(
                                ps[:, soff:soff + sw],
                                lhsT=x8_sb[:, 2 * kp:2 * kp + 2, r0:r0 + 128],
                                rhs=wh8_sb[:, 2 * kp:2 * kp + 2, off + soff:off + soff + sw],
                                start=(kp == 0), stop=(kp == KT // 2 - 1),
                                perf_mode=DR)
                    scale = 1.0 / SW
                else:
                    for soff, sw in _tiles(w, 512):
                        for k in range(KT):
                            nc.tensor.matmul(
                                ps[:, soff:soff + sw],
                                lhsT=xT_sb[:, k, r0:r0 + 128],
                                rhs=wh_sb[:, k, off - F8C + soff:off - F8C + soff + sw],
                                start=(k == 0), stop=(k == KT - 1))
                    scale = 1.0
                if small_chunk:
                    # small chunk: skip the ACT accumulator (its READ_ACCUM
                    # drain costs more than a DVE reduce of the bf16 output)
                    nc.scalar.activation(exph[:, off:off + w], ps[:, :w], EXP,
                                         scale=scale)
                    nc.vector.tensor_reduce(out=partials[:, pcol:pcol + 1],
                                            in_=exph[:, off:off + w],
                                            axis=AX, op=ADD)
                else:
                    nc.scalar.activation(exph[:, off:off + w], ps[:, :w], EXP,
                                         scale=scale,
                                         accum_out=partials[:, pcol:pcol + 1])
                pcol += 1

            # interleave the projection for row-tile m here: m-tile m only
            # needs proj row-tile m//4, emitted for m<4 keeps the PE dense
            if m < n_rt:
                emit_proj(m)

            for ti, (off, w) in enumerate(t0_chunks):
                ps = psum_pool.tile([128, CW], F32, name="ps")
                for soff, sw in _tiles(w, 512):
                    nc.tensor.matmul(ps[:, soff:soff + sw],
                                     lhsT=h0T8_sb[:, :, r0:r0 + 128],
                                     rhs=w0_sb[:, :, off + soff:off + soff + sw],
                                     start=True, stop=True, perf_mode=DR)
                if ti == n0 - 1:
                    nc.scalar.activation(expt0[:, off:off + w], ps[:, :w], EXP)
                    nc.vector.tensor_reduce(out=partials[:, pcol:pcol + 1],
                                            in_=expt0[:, off:off + w],
                                            axis=AX, op=ADD)
                else:
                    nc.scalar.activation(expt0[:, off:off + w], ps[:, :w], EXP,
                                         accum_out=partials[:, pcol:pcol + 1])
                pcol += 1
            for ti, (off, w) in enumerate(t1_chunks):
                ps = psum_pool.tile([128, CW], F32, name="ps")
                for soff, sw in _tiles(w, 512):
                    nc.tensor.matmul(ps[:, soff:soff + sw],
                                     lhsT=h1T_sb[:, r0:r0 + 128],
                                     rhs=w1_sb[:, off + soff:off + soff + sw],
                                     start=True, stop=True)
                if ti == n1 - 1:
                    nc.scalar.activation(expt1[:, off:off + w], ps[:, :w], EXP)
                    nc.vector.tensor_reduce(out=partials[:, pcol:pcol + 1],
                                            in_=expt1[:, off:off + w],
                                            axis=AX, op=ADD)
                else:
                    nc.scalar.activation(expt1[:, off:off + w], ps[:, :w], EXP,
                                         accum_out=partials[:, pcol:pcol + 1])
                pcol += 1

            # per-m-tile sums (DVE, off the PE critical path):
            # head partial includes the 2 replicated cluster exps (they sit
            # inside the last head chunk); rescale them to 1/8 so the 8-way
            # AllGather-sum reconstructs them exactly once.
            csum = small.tile([128, 1], F32, name="csum")
            nc.vector.tensor_reduce(out=csum[:], in_=exph[:, HEAD_SLICE:HEAD_COLS],
                                    axis=AX, op=ADD)
            hraw = small.tile([128, 1], F32, name="hraw")
            nc.vector.tensor_reduce(out=hraw[:], in_=partials[:, 0:nh],
                                    axis=AX, op=ADD)
            c78 = small.tile([128, 1], F32, name="c78")
            nc.vector.tensor_scalar_mul(c78[:], csum[:], -(N_CORES - 1.0) / N_CORES)
            nc.vector.tensor_add(sums_blk[:, j3:j3 + 1], hraw[:], c78[:])
            nc.vector.tensor_reduce(out=sums_blk[:, j3 + 1:j3 + 2],
                                    in_=partials[:, nh:nh + n0], axis=AX, op=ADD)
            t1raw = small.tile([128, 1], F32, name="t1raw")
            nc.vector.tensor_reduce(out=t1raw[:], in_=partials[:, nh + n0:NPART],
                                    axis=AX, op=ADD)
            nc.vector.tensor_add(sums_blk[:, j3 + 2:j3 + 3], t1raw[:], npad_sb[:])
            per_m[m] = (exph, expt0, expt1)

        def emit_comm(blk, sums_blk):
            # One AllGather per block. Sums travel transposed ([3*bs, 128]
            # rows) so every DMA burst is 512B. cc_in staging rides the
            # scalar queue (HWDGE) so the gpsimd queue holds only the
            # collective issues.
            bs = len(blk)
            sc = bs * 3
            psT = psum_a.tile([3 * 3, 128], F32, name="psT")
            nc.tensor.transpose(psT[:sc, :], sums_blk[:, :sc], ident[:, :])
            sumsT = small.tile([3 * 3, 128], F32, name="sumsT", bufs=2)
            nc.scalar.copy(sumsT[:sc, :], psT[:sc, :])
            cc_in = dram.tile([sc, 128], F32, name=f"cc_in{bs}")
            cc_out = dram.tile([N_CORES * sc, 128], F32, name=f"cc_out{bs}",
                               addr_space="Shared")
            nc.scalar.dma_start(out=cc_in[:, :], in_=sumsT[:sc, :])
            nc.gpsimd.collective_compute(
                "AllGather", mybir.AluOpType.bypass,
                replica_groups=[list(range(N_CORES))],
                ins=[cc_in.opt()], outs=[cc_out.opt()],
            )
            return cc_out

        # stores ride the sync queue: a store's semaphore wait (on the
        # DVE scale-mul producing its staging tile) would otherwise block
        # subsequent exp work on the Act queue, stalling PSUM drain -> PE.
        # (gpsimd is used as a second store queue only in the final return,
        # when no further collective needs its queue.)
        store_q = [nc.sync, nc.gpsimd]

        def emit_return(blk, per_m, cc_out, final=False):
            bs = len(blk)
            sc = bs * 3
            gsall = small.tile([N_CORES * sc, 128], F32, name=f"gsall{bs}", bufs=2)
            nc.sync.dma_start(out=gsall[:, :], in_=cc_out[:, :])
            gstp = psum_g.tile([128, N_CORES * 3 * 3], F32, name="gstp")
            nc.tensor.transpose(gstp[:, :N_CORES * sc], gsall[:, :],
                                ident[:N_CORES * sc, :N_CORES * sc])
            # free layout of gstp: (rank r, m-index jj, col c) -> r*sc + jj*3 + c
            gst_v = gstp[:, :N_CORES * sc].rearrange("p (r s) -> p s r", r=N_CORES)
            for jj, m in enumerate(blk):
                exph, expt0, expt1 = per_m[m]
                r0 = m * 128
                # gs: [head_den, t0_den, t1_den] (cluster exps already folded
                # into head_den on the compute side); read directly from PSUM
                gs = small.tile([128, 3], F32, name="gs")
                nc.vector.tensor_reduce(out=gs[:, :],
                                        in_=gst_v[:, jj * 3:(jj + 1) * 3, :],
                                        axis=AX, op=ADD)
                rec = small.tile([128, 3], F32, name="rec")
                nc.vector.reciprocal(rec[:, :], gs[:, :])
                u = small.tile([128, 2], F32, name="u")
                nc.vector.tensor_scalar_mul(u[:, :], exph[:, HEAD_SLICE:HEAD_COLS],
                                            rec[:, 0:1])
                ts = small.tile([128, 2], F32, name="ts")
                nc.vector.tensor_mul(ts[:, :], u[:, :], rec[:, 1:3])

                # scale + store in wide staging tiles (big contiguous DMA bursts)
                sections = [(0, HEAD_SLICE, exph, rec[:, 0:1]),
                            (HEAD_SLICE, T0_SLICE, expt0, ts[:, 0:1]),
                            (HEAD_SLICE + T0_SLICE, T1_SLICE, expt1, ts[:, 1:2])]
                for ci, (soff, sw) in enumerate(out_chunks):
                    # in the tail returns there is no more exp/AG work, so
                    # alternate chunks between mul engines (Act vs DVE) and
                    # store queues (gpsimd vs sync): two parallel drains
                    on_act = final and (jj * 3 + ci) % 2 == 0
                    sq = store_q[1] if on_act else store_q[0]
                    ot = outp.tile([128, CH], BF16, name="ot")
                    for (base, slen, exp_t, scale_ap) in sections:
                        lo = max(soff, base)
                        hi = min(soff + sw, base + slen)
                        if lo >= hi:
                            continue
                        if on_act:
                            nc.scalar.activation(
                                ot[:, lo - soff:hi - soff],
                                exp_t[:, lo - base:hi - base], COPY,
                                scale=scale_ap)
                        else:
                            nc.vector.tensor_scalar_mul(
                                ot[:, lo - soff:hi - soff],
                                exp_t[:, lo - base:hi - base], scale_ap)
                    sq.dma_start(
                        out=out_ext[r0:r0 + 128, soff:soff + sw], in_=ot[:, :sw])

        # blocks of 3 m-tiles tapering to [2,2]: an AG posts ~6us after its
        # block's sums and runs 10-17us on the serial CC engine, so the
        # second-to-last AG (m12-13) completes during m14-15's compute and
        # only the last 2-m-tile AG + its dual-engine return stay exposed.
        blocks = []
        rem = 0
        while m_tiles - rem > 4:
            blocks.append(list(range(rem, rem + 3)))
            rem += 3
        while rem < m_tiles:
            blocks.append(list(range(rem, rem + 2)))
            rem += 2

        # Software-pipelined: block b's AllGather is issued right after its
        # compute; block b-1's return (gather fetch, transpose, normalize,
        # store) is emitted after the SECOND m-tile of block b+1, giving the
        # AllGather ~1.6 blocks of slack before the in-order PE hits the
        # gather-transpose that waits on it.
        pending = None
        for bi, blk in enumerate(blocks):
            per_m = {}
            sums_blk = small.tile([128, 3 * 3], F32, name="sums_blk")
            for j, m in enumerate(blk):
                emit_compute(m, j, sums_blk, per_m)
                if j == 1 and pending is not None:
                    emit_return(*pending)
                    pending = None
            cc_out = emit_comm(blk, sums_blk)
            if pending is not None:
                emit_return(*pending)
                pending = None
            pending = (blk, per_m, cc_out)
        emit_return(*pending, final=True)

    nc.compile()
    return nc


def _shard_inputs(x2d, head_weight, tail_proj_0, tail_w_0, tail_proj_1, tail_w_1):
    bf = ml_dtypes.bfloat16
    f8 = ml_dtypes.float8_e4m3fn
    rows = x2d.shape[0]

    def k3d(a, dtype):
        # [D, C] -> [128, KT, C] with (k p) row split
        return np.ascontiguousarray(
            a.reshape(KT, 128, a.shape[1]).transpose(1, 0, 2)).astype(dtype)

    xT = np.ascontiguousarray(x2d.T)          # [D, rows]
    cluster = head_weight[:, B0:B0 + 2]
    w1p = np.zeros((P1, T1_PADDED), np.float32)
    w1p[:, :T1_ALL] = tail_w_1
    xTb = k3d(xT, bf)
    xT8 = k3d(xT, f8)
    p08 = k3d(tail_proj_0 * 16.0, f8)
    p18 = k3d(tail_proj_1 * 16.0, f8)
    in_maps = []
    for c in range(N_CORES):
        wh = np.concatenate(
            [head_weight[:, c * HEAD_SLICE:(c + 1) * HEAD_SLICE], cluster], axis=1)
        w0c = np.ascontiguousarray(
            tail_w_0[:, c * T0_SLICE:(c + 1) * T0_SLICE] * 8.0)
        npad = -float(T1_PADDED - T1_ALL) if c == N_CORES - 1 else 0.0
        in_maps.append({
            "xT": xTb,
            "x8": xT8,
            "wh": k3d(np.ascontiguousarray(wh[:, F8C:]), bf),
            "wh8": k3d(np.ascontiguousarray(wh[:, :F8C] * SW), f8),
            "p0": p08,
            "w0": np.ascontiguousarray(
                w0c.reshape(P0 // 128, 128, T0_SLICE).transpose(1, 0, 2)).astype(f8),
            "p1": p18,
            "w1": np.ascontiguousarray(
                w1p[:, c * T1_SLICE:(c + 1) * T1_SLICE]).astype(bf),
            "negpad": np.full((128, 1), npad, np.float32),
        })
    return in_maps


def _assemble(outs, rows):
    full = np.empty((rows, V), np.float32)
    for c in range(N_CORES):
        o = np.asarray(outs[c]).astype(np.float32)
        full[:, c * HEAD_SLICE:(c + 1) * HEAD_SLICE] = o[:, :HEAD_SLICE]
        full[:, B0 + c * T0_SLICE:B0 + (c + 1) * T0_SLICE] = \
            o[:, HEAD_SLICE:HEAD_SLICE + T0_SLICE]
        lo = c * T1_SLICE
        hi = min((c + 1) * T1_SLICE, T1_ALL)
        full[:, B0 + T0_ALL + lo:B0 + T0_ALL + hi] = \
            o[:, HEAD_SLICE + T0_SLICE:HEAD_SLICE + T0_SLICE + (hi - lo)]
    return full


RUN_KWARGS = {}      # test harness may set e.g. {"trace": True}
LAST_RESULT = None   # test harness reads exec_time_ns / profile from here


def kernel(x, head_weight, tail_proj_0, tail_w_0, tail_proj_1, tail_w_1):
    global LAST_RESULT
    x = np.asarray(x, dtype=np.float32)
    n, t, d = x.shape
    rows = n * t
    nc = build(rows)
    in_maps = _shard_inputs(
        x.reshape(rows, d),
        np.asarray(head_weight, dtype=np.float32),
        np.asarray(tail_proj_0, dtype=np.float32),
        np.asarray(tail_w_0, dtype=np.float32),
        np.asarray(tail_proj_1, dtype=np.float32),
        np.asarray(tail_w_1, dtype=np.float32),
    )
    res = run_bass_kernel_spmd(nc, in_maps, core_ids=list(range(N_CORES)),
                               **RUN_KWARGS)
    LAST_RESULT = res
    full = _assemble([r["out"] for r in res.results], rows)
    return full.reshape(n, t, V)


# revision 20
# speedup vs baseline: 1.0359x; 1.0359x over previous
"""Trainium2 Bass kernel: adaptive-input softmax ('softmax' mode), 8 NeuronCores.

Strategy: vocab tensor-parallel. Each core owns a 1/8 slice of the head token
columns (2500 of 20000), tail0 columns (2500 of 20000) and tail1 columns
(1283 of ceil(10257/8)*8, zero-padded), computes partition-local logits ->
exp, and the per-row softmax denominators are completed with a small
cross-core AllGather of per-row exp-sums (overlapped with compute).
The 2 cluster logits are computed replicated on every core inside the wide
head exp; their contribution is rescaled (x1/8) on the DVE before the
AllGather so the 8-way sum reconstructs them exactly once.

Precision plan: the head matmul (99.99% of output probability mass) runs in
bf16. The tail projections and the tail0 logit matmul run in fp8e4m3 with
DoubleRow perf mode; the tails carry ~1e-4 of the probability mass so their
~1% internal error is invisible in the rel-l2 metric. Scale management:
host stores p0*16, p1*16, w0*8 in fp8; the kernel stores h0/8 in fp8 so
h0/8 @ (8*w0) reproduces h0@w0 exactly. Output stores in bf16.

v2 perf structure (from the v1 trace):
- PSUM chunks are 1024 wide (2 banks) with a 3-deep ring: 8 exp ACTIVATEs
  per m-tile instead of 13 -> ~70us less ACT time (ACTIVATE overhead +
  ACTIVATION_READ_ACCUMULATOR drains).
- Loads are ~20 coarse DMAs (3D dram params) in first-use order across the
  sync/scalar/gpsimd queues instead of ~130 fine ones; first matmul ~5us.
- Block sizes [3,3,3,3,2,1,1]: the last AllGathers carry 1 m-tile each, so
  after the final matmul only one small AG (~12us) + one m-tile of
  normalize+store remains.
- proj PSUM->SBUF copies ride the DVE, keeping ACT for exps.
"""
import numpy as np
import ml_dtypes
from contextlib import ExitStack

import concourse.bass as bass
import concourse.tile as tile
from concourse import bacc, mybir
from concourse.bass_utils import run_bass_kernel_spmd
from concourse.masks import make_identity

N_CORES = 8
D = 1024
KT = D // 128                      # contraction k-tiles over D
B0 = 20000                         # head token columns
HEAD_SLICE = B0 // N_CORES         # 2500 per core
T0_ALL = 20000
T0_SLICE = T0_ALL // N_CORES       # 2500 per core
T1_ALL = 10257
T1_SLICE = -(-T1_ALL // N_CORES)   # 1283 per core (global pad to 10264)
T1_PADDED = T1_SLICE * N_CORES
HEAD_COLS = HEAD_SLICE + 2         # + 2 replicated cluster columns
OUT_COLS = HEAD_SLICE + T0_SLICE + T1_SLICE   # 6283 per-core output columns
P0 = 256                           # tail0 projection dim
P1 = 64                            # tail1 projection dim
V = B0 + T0_ALL + T1_ALL           # 50257
SW = 16.0                          # head-weight fp8 pre-scale (tails)
CW = 1024                          # psum chunk width (2 banks)
F8C = 1024                         # head cols per core computed in fp8 (DR)
BF_COLS = HEAD_COLS - F8C          # remaining bf16 head cols (incl clusters)

BF16 = mybir.dt.bfloat16
F32 = mybir.dt.float32
F8 = mybir.dt.float8e4
DR = mybir.MatmulPerfMode.DoubleRow
EXP = mybir.ActivationFunctionType.Exp
COPY = mybir.ActivationFunctionType.Copy
ADD = mybir.AluOpType.add
AX = mybir.AxisListType.X


def _tiles(total, step=512):
    out, off = [], 0
    while off < total:
        w = min(step, total - off)
        out.append((off, w))
        off += w
    return out


def build(rows):
    assert rows % 512 == 0
    m_tiles = rows // 128

    nc = bacc.Bacc("TRN2", target_bir_lowering=False, debug=False,
                   num_devices=N_CORES)
    # 3D layouts so each logical tensor loads in one (or per-k) DMA.
    xT_ext = nc.declare_dram_parameter("xT", [128, KT, rows], BF16, isOutput=False)
    x8_ext = nc.declare_dram_parameter("x8", [128, KT, rows], F8, isOutput=False)
    wh_ext = nc.declare_dram_parameter("wh", [128, KT, BF_COLS], BF16, isOutput=False)
    wh8_ext = nc.declare_dram_parameter("wh8", [128, KT, F8C], F8, isOutput=False)
    p0_ext = nc.declare_dram_parameter("p0", [128, KT, P0], F8, isOutput=False)
    w0_ext = nc.declare_dram_parameter("w0", [128, P0 // 128, T0_SLICE], F8, isOutput=False)
    p1_ext = nc.declare_dram_parameter("p1", [128, KT, P1], F8, isOutput=False)
    w1_ext = nc.declare_dram_parameter("w1", [P1, T1_SLICE], BF16, isOutput=False)
    npad_ext = nc.declare_dram_parameter("negpad", [128, 1], F32, isOutput=False)
    out_ext = nc.declare_dram_parameter("out", [rows, OUT_COLS], BF16, isOutput=True)

    # head chunk 0 (F8C cols) runs fp8+DoubleRow; the rest is bf16
    head_chunks = _tiles(HEAD_COLS, CW)   # (0,1024),(1024,1024),(2048,454)
    t0_chunks = _tiles(T0_SLICE, CW)      # (0,1024),(1024,1024),(2048,452)
    t1_chunks = _tiles(T1_SLICE, CW)      # (0,1024),(1024,259)
    nh, n0, n1 = len(head_chunks), len(t0_chunks), len(t1_chunks)
    NPART = nh + n0 + n1                  # partial-sum columns per m-tile
    proj_tiles = _tiles(rows, 512)
    out_chunks = _tiles(OUT_COLS, -(-OUT_COLS // 3))   # 3 even-ish store chunks
    CH = out_chunks[0][1]

    with ExitStack() as ctx:
        tc = ctx.enter_context(tile.TileContext(nc))
        const = ctx.enter_context(tc.tile_pool(name="const", bufs=1))
        psum_pool = ctx.enter_context(tc.tile_pool(name="psum", bufs=3, space="PSUM"))
        psum_a = ctx.enter_context(tc.tile_pool(name="psum_a", bufs=1, space="PSUM"))
        psum_g = ctx.enter_context(tc.tile_pool(name="psum_g", bufs=1, space="PSUM"))
        exp_pool = ctx.enter_context(tc.tile_pool(name="exppool", bufs=6))
        outp = ctx.enter_context(tc.tile_pool(name="outp", bufs=3))
        small = ctx.enter_context(tc.tile_pool(name="small", bufs=7))
        dram = ctx.enter_context(tc.tile_pool(name="dram", bufs=3, space="DRAM"))

        # ---------- resident inputs ----------
        xT_sb = const.tile([128, KT, rows], BF16, name="xT_sb")
        x8_sb = const.tile([128, KT, rows], F8, name="x8_sb")
        wh_sb = const.tile([128, KT, BF_COLS], BF16, name="wh_sb")
        wh8_sb = const.tile([128, KT, F8C], F8, name="wh8_sb")
        p0_sb = const.tile([128, KT, P0], F8, name="p0_sb")
        p1_sb = const.tile([128, KT, P1], F8, name="p1_sb")
        w0_sb = const.tile([128, P0 // 128, T0_SLICE], F8, name="w0_sb")
        w1_sb = const.tile([P1, T1_SLICE], BF16, name="w1_sb")
        npad_sb = const.tile([128, 1], F32, name="npad_sb")

        # warm-up AllGather first, with no dependencies at all, so the
        # ~40us first-collective init burns on the CC engine starting at
        # t~1us, fully inside the load phase
        warm_sb = small.tile([1, 128], F32, name="warm_sb")
        nc.gpsimd.memset(warm_sb[:, :], 0.0)
        warm_in = dram.tile([1, 128], F32, name="warm_in")
        warm_out = dram.tile([N_CORES, 128], F32, name="warm_out",
                             addr_space="Shared")
        nc.gpsimd.dma_start(out=warm_in[:, :], in_=warm_sb[:, :])
        nc.gpsimd.collective_compute(
            "AllGather", mybir.AluOpType.bypass,
            replica_groups=[list(range(N_CORES))],
            ins=[warm_in.opt()], outs=[warm_out.opt()],
        )

        # ---------- loads: one HWDGE queue, strict first-use order ----------
        # A single logical DMA queue drains roughly in order, so transfers
        # complete in the order compute consumes them: m0's fp8 head chunk
        # first, then the bf16 head weights per-k, then tail weights, then
        # the remaining x row-chunks (needed from m-tile 4 on).
        nc.sync.dma_start(out=x8_sb[:, :, 0:512], in_=x8_ext[:, :, 0:512])
        nc.sync.dma_start(out=wh8_sb[:, :, :], in_=wh8_ext[:, :, :])
        nc.sync.dma_start(out=xT_sb[:, :, 0:512], in_=xT_ext[:, :, 0:512])
        for k in range(KT):
            nc.sync.dma_start(out=wh_sb[:, k, :], in_=wh_ext[:, k, :])
        nc.sync.dma_start(out=p0_sb[:, :, :], in_=p0_ext[:, :, :])
        nc.sync.dma_start(out=p1_sb[:, :, :], in_=p1_ext[:, :, :])
        nc.sync.dma_start(out=w0_sb[:, :, :], in_=w0_ext[:, :, :])
        nc.sync.dma_start(out=w1_sb[:, :], in_=w1_ext[:, :])
        nc.sync.dma_start(out=npad_sb[:], in_=npad_ext[:])
        for roff in range(512, rows, 512):
            nc.sync.dma_start(out=xT_sb[:, :, roff:roff + 512],
                              in_=xT_ext[:, :, roff:roff + 512])
            nc.sync.dma_start(out=x8_sb[:, :, roff:roff + 512],
                              in_=x8_ext[:, :, roff:roff + 512])

        ident = const.tile([128, 128], F32, name="ident")
        make_identity(nc, ident)

        # ---------- tail hidden projections (fp8 DoubleRow) ----------
        # h0T8 holds h0/8 in fp8: psum = x8 @ (16 p0) = 16 h0, DVE scale
        # 1/128 -> h0/8.  h1T holds h1 in bf16 (psum = 16 h1, scale 1/16).
        h0T8_sb = const.tile([128, P0 // 128, rows], F8, name="h0T8_sb")
        h1T_sb = const.tile([P1, rows], BF16, name="h1T_sb")

        def emit_proj(rt):
            roff, rw = proj_tiles[rt]
            for mp in range(P0 // 128):
                ps = psum_pool.tile([128, CW], F32, name="ps")
                for kp in range(KT // 2):
                    nc.tensor.matmul(ps[:, :rw],
                                     lhsT=p0_sb[:, 2 * kp:2 * kp + 2, mp * 128:(mp + 1) * 128],
                                     rhs=x8_sb[:, 2 * kp:2 * kp + 2, roff:roff + rw],
                                     start=(kp == 0), stop=(kp == KT // 2 - 1),
                                     perf_mode=DR)
                nc.vector.tensor_scalar_mul(h0T8_sb[:, mp, roff:roff + rw],
                                            ps[:, :rw], 1.0 / 128.0)
            ps = psum_pool.tile([128, CW], F32, name="ps")
            for kp in range(KT // 2):
                nc.tensor.matmul(ps[:P1, :rw],
                                 lhsT=p1_sb[:, 2 * kp:2 * kp + 2, :],
                                 rhs=x8_sb[:, 2 * kp:2 * kp + 2, roff:roff + rw],
                                 start=(kp == 0), stop=(kp == KT // 2 - 1),
                                 perf_mode=DR)
            nc.vector.tensor_scalar_mul(h1T_sb[:, roff:roff + rw],
                                        ps[:P1, :rw], 1.0 / 16.0)

        # ---------- per m-tile compute (head -> proj -> tails) ----------
        n_rt = len(proj_tiles)

        def emit_t1_chunk(st, ci):
            # one tail1 psum chunk for a (possibly previous) m-tile
            (m, expt1, partials, sums_blk, j3) = st
            r0 = m * 128
            off, w = t1_chunks[ci]
            ps = psum_pool.tile([128, CW], F32, name="ps")
            for soff, sw in _tiles(w, 512):
                nc.tensor.matmul(ps[:, soff:soff + sw],
                                 lhsT=h1T_sb[:, r0:r0 + 128],
                                 rhs=w1_sb[:, off + soff:off + soff + sw],
                                 start=True, stop=True)
            if ci == n1 - 1:
                nc.scalar.activation(expt1[:, off:off + w], ps[:, :w], EXP)
                nc.vector.tensor_reduce(out=partials[:, nh + n0 + ci:nh + n0 + ci + 1],
                                        in_=expt1[:, off:off + w],
                                        axis=AX, op=ADD)
            else:
                nc.scalar.activation(expt1[:, off:off + w], ps[:, :w], EXP,
                                     accum_out=partials[:, nh + n0 + ci:nh + n0 + ci + 1])

        def emit_t1_sums(st):
            (m, expt1, partials, sums_blk, j3) = st
            t1raw = small.tile([128, 1], F32, name="t1raw")
            nc.vector.tensor_reduce(out=t1raw[:], in_=partials[:, nh + n0:NPART],
                                    axis=AX, op=ADD)
            nc.vector.tensor_add(sums_blk[:, j3 + 2:j3 + 3], t1raw[:], npad_sb[:])

        def emit_compute(m, j, sums_blk, per_m, t1_pending, defer_t1):
            # t1_pending: deferred tail1 state of m-1, interleaved into this
            # m-tile's head phase so its matmuls never wait on the exp of a
            # tail0 chunk that shares their psum ring slot.
            r0 = m * 128
            exph = exp_pool.tile([128, HEAD_COLS], BF16, name="exph")
            expt0 = exp_pool.tile([128, T0_SLICE], BF16, name="expt0")
            expt1 = exp_pool.tile([128, T1_SLICE], BF16, name="expt1")
            partials = small.tile([128, NPART], F32, name="partials")
            j3 = j * 3

            pcol = 0
            for hi, (off, w) in enumerate(head_chunks):
                if t1_pending is not None and hi in (1, 2):
                    emit_t1_chunk(t1_pending, hi - 1)
                    if hi == 2:
                        emit_t1_sums(t1_pending)
                ps = psum_pool.tile([128, CW], F32, name="ps")
                small_chunk = (hi == nh - 1)
                if off < F8C:
                    # fp8 DoubleRow head chunk: psum = x8 @ (16*wh8)
                    assert off + w <= F8C
                    for soff, sw in _tiles(w, 512):
                        for kp in range(KT // 2):
                            nc.tensor.matmul(
                                ps[:, soff:soff + sw],
                                lhsT=x8_sb[:, 2 * kp:2 * kp + 2, r0:r0 + 128],
                                rhs=wh8_sb[:, 2 * kp:2 * kp + 2, off + soff:off + soff + sw],
                                start=(kp == 0), stop=(kp == KT // 2 - 1),
                                perf_mode=DR)
                    scale = 1.0 / SW
                else:
                    for soff, sw in _tiles(w, 512):
                        for k in range(KT):
                            nc.tensor.matmul(
                                ps[:, soff:soff + sw],
                                lhsT=xT_sb[:, k, r0:r0 + 128],
                                rhs=wh_sb[:, k, off - F8C + soff:off - F8C + soff + sw],
                                start=(k == 0), stop=(k == KT - 1))
                    scale = 1.0
                if small_chunk:
                    # small chunk: skip the ACT accumulator (its READ_ACCUM
                    # drain costs more than a DVE reduce of the bf16 output)
                    nc.scalar.activation(exph[:, off:off + w], ps[:, :w], EXP,
                                         scale=scale)
                    nc.vector.tensor_reduce(out=partials[:, pcol:pcol + 1],
                                            in_=exph[:, off:off + w],
                                            axis=AX, op=ADD)
                else:
                    nc.scalar.activation(exph[:, off:off + w], ps[:, :w], EXP,
                                         scale=scale,
                                         accum_out=partials[:, pcol:pcol + 1])
                pcol += 1

            # interleave the projection for row-tile m here: m-tile m only
            # needs proj row-tile m//4 (row-tile 0 is emitted pre-loop to
            # fill the initial weight-load window)
            if 0 < m < n_rt:
                emit_proj(m)

            for ti, (off, w) in enumerate(t0_chunks):
                ps = psum_pool.tile([128, CW], F32, name="ps")
                for soff, sw in _tiles(w, 512):
                    nc.tensor.matmul(ps[:, soff:soff + sw],
                                     lhsT=h0T8_sb[:, :, r0:r0 + 128],
                                     rhs=w0_sb[:, :, off + soff:off + soff + sw],
                                     start=True, stop=True, perf_mode=DR)
                if ti == n0 - 1:
                    nc.scalar.activation(expt0[:, off:off + w], ps[:, :w], EXP)
                    nc.vector.tensor_reduce(out=partials[:, pcol:pcol + 1],
                                            in_=expt0[:, off:off + w],
                                            axis=AX, op=ADD)
                else:
                    nc.scalar.activation(expt0[:, off:off + w], ps[:, :w], EXP,
                                         accum_out=partials[:, pcol:pcol + 1])
                pcol += 1
            # per-m-tile head/t0 sums (DVE, off the PE critical path):
            # head partial includes the 2 replicated cluster exps (they sit
            # inside the last head chunk); rescale them to 1/8 so the 8-way
            # AllGather-sum reconstructs them exactly once.
            csum = small.tile([128, 1], F32, name="csum")
            nc.vector.tensor_reduce(out=csum[:], in_=exph[:, HEAD_SLICE:HEAD_COLS],
                                    axis=AX, op=ADD)
            hraw = small.tile([128, 1], F32, name="hraw")
            nc.vector.tensor_reduce(out=hraw[:], in_=partials[:, 0:nh],
                                    axis=AX, op=ADD)
            c78 = small.tile([128, 1], F32, name="c78")
            nc.vector.tensor_scalar_mul(c78[:], csum[:], -(N_CORES - 1.0) / N_CORES)
            nc.vector.tensor_add(sums_blk[:, j3:j3 + 1], hraw[:], c78[:])
            nc.vector.tensor_reduce(out=sums_blk[:, j3 + 1:j3 + 2],
                                    in_=partials[:, nh:nh + n0], axis=AX, op=ADD)
            per_m[m] = (exph, expt0, expt1)

            st = (m, expt1, partials, sums_blk, j3)
            if defer_t1:
                return st
            for ci in range(n1):
                emit_t1_chunk(st, ci)
            emit_t1_sums(st)
            return None

        def emit_comm(blk, sums_blk):
            # One AllGather per block. Sums travel transposed ([3*bs, 128]
            # rows) so every DMA burst is 512B. cc_in staging rides the
            # scalar queue (HWDGE) so the gpsimd queue holds only the
            # collective issues.
            bs = len(blk)
            sc = bs * 3
            psT = psum_a.tile([3 * 3, 128], F32, name="psT")
            nc.tensor.transpose(psT[:sc, :], sums_blk[:, :sc], ident[:, :])
            sumsT = small.tile([3 * 3, 128], F32, name="sumsT", bufs=2)
            nc.scalar.copy(sumsT[:sc, :], psT[:sc, :])
            cc_in = dram.tile([sc, 128], F32, name=f"cc_in{bs}")
            cc_out = dram.tile([N_CORES * sc, 128], F32, name=f"cc_out{bs}",
                               addr_space="Shared")
            nc.scalar.dma_start(out=cc_in[:, :], in_=sumsT[:sc, :])
            nc.gpsimd.collective_compute(
                "AllGather", mybir.AluOpType.bypass,
                replica_groups=[list(range(N_CORES))],
                ins=[cc_in.opt()], outs=[cc_out.opt()],
            )
            return cc_out

        # stores ride the sync queue: a store's semaphore wait (on the
        # DVE scale-mul producing its staging tile) would otherwise block
        # subsequent exp work on the Act queue, stalling PSUM drain -> PE.
        # (gpsimd is used as a second store queue only in the final return,
        # when no further collective needs its queue.)
        store_q = [nc.sync, nc.gpsimd]

        def emit_return(blk, per_m, cc_out, final=False):
            bs = len(blk)
            sc = bs * 3
            gsall = small.tile([N_CORES * sc, 128], F32, name=f"gsall{bs}", bufs=2)
            nc.sync.dma_start(out=gsall[:, :], in_=cc_out[:, :])
            gstp = psum_g.tile([128, N_CORES * 3 * 3], F32, name="gstp")
            nc.tensor.transpose(gstp[:, :N_CORES * sc], gsall[:, :],
                                ident[:N_CORES * sc, :N_CORES * sc])
            # free layout of gstp: (rank r, m-index jj, col c) -> r*sc + jj*3 + c
            gst_v = gstp[:, :N_CORES * sc].rearrange("p (r s) -> p s r", r=N_CORES)
            for jj, m in enumerate(blk):
                exph, expt0, expt1 = per_m[m]
                r0 = m * 128
                # gs: [head_den, t0_den, t1_den] (cluster exps already folded
                # into head_den on the compute side); read directly from PSUM
                gs = small.tile([128, 3], F32, name="gs")
                nc.vector.tensor_reduce(out=gs[:, :],
                                        in_=gst_v[:, jj * 3:(jj + 1) * 3, :],
                                        axis=AX, op=ADD)
                rec = small.tile([128, 3], F32, name="rec")
                nc.vector.reciprocal(rec[:, :], gs[:, :])
                u = small.tile([128, 2], F32, name="u")
                nc.vector.tensor_scalar_mul(u[:, :], exph[:, HEAD_SLICE:HEAD_COLS],
                                            rec[:, 0:1])
                ts = small.tile([128, 2], F32, name="ts")
                nc.vector.tensor_mul(ts[:, :], u[:, :], rec[:, 1:3])

                # scale + store in wide staging tiles (big contiguous DMA bursts)
                sections = [(0, HEAD_SLICE, exph, rec[:, 0:1]),
                            (HEAD_SLICE, T0_SLICE, expt0, ts[:, 0:1]),
                            (HEAD_SLICE + T0_SLICE, T1_SLICE, expt1, ts[:, 1:2])]
                for ci, (soff, sw) in enumerate(out_chunks):
                    # in the tail returns there is no more exp/AG work, so
                    # alternate chunks between mul engines (Act vs DVE) and
                    # store queues (gpsimd vs sync): two parallel drains
                    on_act = final and (jj * 3 + ci) % 2 == 0
                    sq = store_q[1] if on_act else store_q[0]
                    ot = outp.tile([128, CH], BF16, name="ot")
                    for (base, slen, exp_t, scale_ap) in sections:
                        lo = max(soff, base)
                        hi = min(soff + sw, base + slen)
                        if lo >= hi:
                            continue
                        if on_act:
                            nc.scalar.activation(
                                ot[:, lo - soff:hi - soff],
                                exp_t[:, lo - base:hi - base], COPY,
                                scale=scale_ap)
                        else:
                            nc.vector.tensor_scalar_mul(
                                ot[:, lo - soff:hi - soff],
                                exp_t[:, lo - base:hi - base], scale_ap)
                    sq.dma_start(
                        out=out_ext[r0:r0 + 128, soff:soff + sw], in_=ot[:, :sw])

        # blocks of 3 m-tiles tapering to [2,2]: an AG posts ~6us after its
        # block's sums and runs 10-17us on the serial CC engine, so the
        # second-to-last AG (m12-13) completes during m14-15's compute and
        # only the last 2-m-tile AG + its dual-engine return stay exposed.
        blocks = []
        rem = 0
        while m_tiles - rem > 4:
            blocks.append(list(range(rem, rem + 3)))
            rem += 3
        while rem < m_tiles:
            blocks.append(list(range(rem, rem + 2)))
            rem += 2

        # Software-pipelined: block b's AllGather is issued right after its
        # compute; block b-1's return (gather fetch, transpose, normalize,
        # store) is emitted after the SECOND m-tile of block b+1, giving the
        # AllGather ~1.6 blocks of slack before the in-order PE hits the
        # gather-transpose that waits on it.
        emit_proj(0)
        pending = None
        t1_pending = None
        for bi, blk in enumerate(blocks):
            per_m = {}
            sums_blk = small.tile([128, 3 * 3], F32, name="sums_blk")
            for j, m in enumerate(blk):
                t1_pending = emit_compute(m, j, sums_blk, per_m, t1_pending,
                                          defer_t1=(j != len(blk) - 1))
                if j == 1 and pending is not None:
                    emit_return(*pending)
                    pending = None
            cc_out = emit_comm(blk, sums_blk)
            if pending is not None:
                emit_return(*pending)
                pending = None
            pending = (blk, per_m, cc_out)
        emit_return(*pending, final=True)

    nc.compile()
    return nc


def _shard_inputs(x2d, head_weight, tail_proj_0, tail_w_0, tail_proj_1, tail_w_1):
    bf = ml_dtypes.bfloat16
    f8 = ml_dtypes.float8_e4m3fn
    rows = x2d.shape[0]

    def k3d(a, dtype):
        # [D, C] -> [128, KT, C] with (k p) row split
        return np.ascontiguousarray(
            a.reshape(KT, 128, a.shape[1]).transpose(1, 0, 2)).astype(dtype)

    xT = np.ascontiguousarray(x2d.T)          # [D, rows]
    cluster = head_weight[:, B0:B0 + 2]
    w1p = np.zeros((P1, T1_PADDED), np.float32)
    w1p[:, :T1_ALL] = tail_w_1
    xTb = k3d(xT, bf)
    xT8 = k3d(xT, f8)
    p08 = k3d(tail_proj_0 * 16.0, f8)
    p18 = k3d(tail_proj_1 * 16.0, f8)
    in_maps = []
    for c in range(N_CORES):
        wh = np.concatenate(
            [head_weight[:, c * HEAD_SLICE:(c + 1) * HEAD_SLICE], cluster], axis=1)
        w0c = np.ascontiguousarray(
            tail_w_0[:, c * T0_SLICE:(c + 1) * T0_SLICE] * 8.0)
        npad = -float(T1_PADDED - T1_ALL) if c == N_CORES - 1 else 0.0
        in_maps.append({
            "xT": xTb,
            "x8": xT8,
            "wh": k3d(np.ascontiguousarray(wh[:, F8C:]), bf),
            "wh8": k3d(np.ascontiguousarray(wh[:, :F8C] * SW), f8),
            "p0": p08,
            "w0": np.ascontiguousarray(
                w0c.reshape(P0 // 128, 128, T0_SLICE).transpose(1, 0, 2)).astype(f8),
            "p1": p18,
            "w1": np.ascontiguousarray(
                w1p[:, c * T1_SLICE:(c + 1) * T1_SLICE]).astype(bf),
            "negpad": np.full((128, 1), npad, np.float32),
        })
    return in_maps


def _assemble(outs, rows):
    full = np.empty((rows, V), np.float32)
    for c in range(N_CORES):
        o = np.asarray(outs[c]).astype(np.float32)
        full[:, c * HEAD_SLICE:(c + 1) * HEAD_SLICE] = o[:, :HEAD_SLICE]
        full[:, B0 + c * T0_SLICE:B0 + (c + 1) * T0_SLICE] = \
            o[:, HEAD_SLICE:HEAD_SLICE + T0_SLICE]
        lo = c * T1_SLICE
        hi = min((c + 1) * T1_SLICE, T1_ALL)
        full[:, B0 + T0_ALL + lo:B0 + T0_ALL + hi] = \
            o[:, HEAD_SLICE + T0_SLICE:HEAD_SLICE + T0_SLICE + (hi - lo)]
    return full


RUN_KWARGS = {}      # test harness may set e.g. {"trace": True}
LAST_RESULT = None   # test harness reads exec_time_ns / profile from here


def kernel(x, head_weight, tail_proj_0, tail_w_0, tail_proj_1, tail_w_1):
    global LAST_RESULT
    x = np.asarray(x, dtype=np.float32)
    n, t, d = x.shape
    rows = n * t
    nc = build(rows)
    in_maps = _shard_inputs(
        x.reshape(rows, d),
        np.asarray(head_weight, dtype=np.float32),
        np.asarray(tail_proj_0, dtype=np.float32),
        np.asarray(tail_w_0, dtype=np.float32),
        np.asarray(tail_proj_1, dtype=np.float32),
        np.asarray(tail_w_1, dtype=np.float32),
    )
    res = run_bass_kernel_spmd(nc, in_maps, core_ids=list(range(N_CORES)),
                               **RUN_KWARGS)
    LAST_RESULT = res
    full = _assemble([r["out"] for r in res.results], rows)
    return full.reshape(n, t, V)
